# revision 14
# baseline (speedup 1.0000x reference)
"""Conv2d 3x3 (stride 1, pad 1) on Trainium2, data-parallel over batch.

Full problem: x [16, 32, 512, 512] f32, kernels [32, 32, 3, 3] f32
-> out [16, 32, 512, 512] f32.

Sharding: batch 16 / 8 cores = 2 images per core; kernels replicated.
No collectives needed.

Per-core kernel strategy (memory-bound target):
- Host casts x and weights to bf16 (tol 2e-2; measured ~2e-3 rel err),
  halving both DMA directions. Output is written bf16 and upcast on host.
- Conv expressed as 9 accumulating 32x32 matmuls (one per tap) into PSUM.
- The 128x128 PE array is addressed as 16 concurrent 32x32 sub-arrays via
  tile_position: row group i = image band i (4 horizontal bands), col
  group j = output-row slot j.
- No padded input layout: width-edge taps (kw != 1) run as width-511
  matmuls into a column-shifted PSUM slice. The first tap is a kw=1
  full-width matmul with start=True so every PSUM element is initialized.
  This keeps every DMA fully contiguous per channel (large descriptors).
- Row mapping m = S*j + s: PSUM col group j holds output rows
  [row0+S*j, row0+S*j+S), so each output DMA writes S consecutive image
  rows = S KiB contiguous runs per channel.
"""

import numpy as np
from contextlib import ExitStack

import concourse.bass as bass
import concourse.mybir as mybir
import concourse.tile as tile
from concourse.bass_utils import run_bass_kernel_spmd

F32 = mybir.dt.float32
BF16 = mybir.dt.bfloat16
NP_BF16 = mybir.dt.np(BF16)
KH = KW = 3
NBANDS = 4  # row groups = horizontal image bands
# Col groups: only 0..2 (PE quadrant 3 / xdata[3] is broken in HW; using
# col group 3 more than halves sustained matmul throughput - measured).
NCOLG = 3

# Taps ordered so the first is kw=1 (full width, start=True initializes
# every PSUM column); kw=0/2 taps only touch a width-511 slice.
TAPS = [(0, 1), (1, 1), (2, 1), (0, 0), (1, 0), (2, 0), (0, 2), (1, 2), (2, 2)]

# Full-problem geometry (hardcoded; kernel.py must be self-contained)
FULL_B, FULL_C, FULL_H, FULL_W = 16, 32, 512, 512
N_CORES = 8


def split_multi_waits(nc, cap=1):
    """This walrus build rejects instructions carrying more than `cap` sync
    wait commands ("Too many sync wait commands", setupSyncWait). Hoist
    excess waits onto single-wait NoOps inserted just before the instruction
    on the same engine queue (queues are in-order, so semantics are
    unchanged)."""
    n_split = 0
    for fn in nc.m.functions:
        for blk in fn.blocks:
            insts = blk.instructions
            if not any(
                i.sync_info is not None and len(i.sync_info.on_wait) > cap
                for i in insts
            ):
                continue
            new = []
            for inst in insts:
                si = inst.sync_info
                if si is not None and len(si.on_wait) > cap:
                    waits = list(si.on_wait)
                    n_split += 1
                    for k in range(0, len(waits) - cap, cap):
                        nop = mybir.InstNoOp(
                            name=nc.get_next_instruction_name(), ins=[], outs=[]
                        )
                        nop.engine = inst.engine
                        nop.sync_info = mybir.SyncInfo(
                            on_wait=waits[k : k + cap], on_update=[]
                        )
                        new.append(nop)
                    inst.sync_info = mybir.SyncInfo(
                        on_wait=waits[len(waits) - cap :],
                        on_update=list(si.on_update),
                    )
                new.append(inst)
            blk.instructions = new
    return n_split


def emit_conv(
    nc, tc, ctx, x_ap, w_ap, out_ap, B, C, H, W, R=32, prefix="", do_mm=True
):
    """Emit the Tile program for a per-core conv: x [B, C, H, W] bf16
    (local batch), w [C, KH*KW*C] bf16 (pre-transposed on host:
    [ic, (kh kw oc)]), out [B, C, H, W] bf16."""
    assert C == 32
    HB = H // NBANDS  # rows per band
    assert HB * NBANDS == H
    assert HB % R == 0
    T = HB // R  # rounds per image
    # Split the R rows of a round across the 3 usable col groups as
    # contiguous blocks of lens[j] rows (group j owns rows
    # [starts[j], starts[j]+lens[j]) of the round).
    base_len = R // NCOLG
    rem = R - base_len * NCOLG
    lens = [base_len + (1 if j < rem else 0) for j in range(NCOLG)]
    starts = [sum(lens[:j]) for j in range(NCOLG)]
    S = lens[0]  # steps per round (group j idles for s >= lens[j])
    RH = R + 2  # input rows needed per round (1-row halo on each side)

    w_pool = ctx.enter_context(tc.tile_pool(name=prefix + "wpool", bufs=1))
    in_pool = ctx.enter_context(tc.tile_pool(name=prefix + "inpool", bufs=3))
    out_pool = ctx.enter_context(tc.tile_pool(name=prefix + "outpool", bufs=2 * NBANDS))
    psum_pool = ctx.enter_context(
        tc.tile_pool(name=prefix + "psumpool", bufs=2 * NBANDS, space="PSUM")
    )

    # Weights: replicate [32, 9*32] into each of the 4 partition groups so
    # lhsT.base_partition() matches the rhs row group.
    w_tile = w_pool.tile([128, KH * KW * C], BF16, name=prefix + "w_tile", tag="w")
    for r in range(NBANDS):
        nc.sync.dma_start(out=w_tile[32 * r : 32 * r + 32, :], in_=w_ap[:, :])

    for b in range(B):
        for t in range(T):
            # ---- load input rows for this round: band i covers output rows
            # [i*HB + t*R, i*HB + t*R + R), needing input rows -1..R+1
            # around it. Packed layout: slot r, col x contiguous.
            in_tile = in_pool.tile(
                [128, RH * W], BF16, name=f"{prefix}in_{b}_{t}", tag="xin"
            )
            in_rows = in_tile.rearrange("p (r w) -> p r w", w=W)
            for i in range(NBANDS):
                row0 = i * HB + t * R
                lo = max(row0 - 1, 0)
                hi = min(row0 + R + 1, H)
                slot0 = lo - (row0 - 1)
                cnt = hi - lo
                dst = in_rows[32 * i : 32 * i + 32, slot0 : slot0 + cnt, :]
                nc.sync.dma_start(out=dst, in_=x_ap[b, :, lo:hi, :])
                if row0 == 0:  # top image boundary: zero row slot 0
                    nc.vector.memset(in_rows[32 * i : 32 * i + 32, 0:1, :], 0.0)
                if row0 + R == H:  # bottom image boundary: zero last slot
                    nc.vector.memset(
                        in_rows[32 * i : 32 * i + 32, RH - 1 : RH, :], 0.0
                    )

            out_tiles = []
            for i in range(NBANDS):
                ot = out_pool.tile(
                    [128, S * W], BF16, name=f"{prefix}out_{b}_{t}_{i}", tag="osb"
                )
                if do_mm != 1:  # timing probe: mark tile written
                    nc.vector.memset(ot[:, 0:1], 0.0)
                out_tiles.append(ot)

            for s in range(S if do_mm else 0):
                jact = [j for j in range(NCOLG) if s < lens[j]]
                psums = {}
                for i in range(NBANDS):
                    pt = psum_pool.tile(
                        [128, W], F32, name=f"{prefix}ps_{b}_{t}_{s}_{i}", tag="acc"
                    )
                    psums[i] = pt
                # 9 taps; 12 concurrent 32x32 sub-array matmuls per tap
                for ti, (kh, kw) in enumerate(TAPS):
                    off = kh * KW + kw
                    for i in range(NBANDS):
                        lhsT = w_tile[32 * i : 32 * i + 32, off * C : off * C + C]
                        for j in jact:
                            m = starts[j] + s  # local output row
                            base = (m + kh) * W  # input row slot m+kh
                            if kw == 1:
                                rhs = in_tile[32 * i : 32 * i + 32, base : base + W]
                                dst = psums[i][32 * j : 32 * j + 32, 0:W]
                            elif kw == 0:
                                rhs = in_tile[
                                    32 * i : 32 * i + 32, base : base + W - 1
                                ]
                                dst = psums[i][32 * j : 32 * j + 32, 1:W]
                            else:  # kw == 2
                                rhs = in_tile[
                                    32 * i : 32 * i + 32, base + 1 : base + W
                                ]
                                dst = psums[i][32 * j : 32 * j + 32, 0 : W - 1]
                            nc.tensor.matmul(
                                dst,
                                lhsT,
                                rhs,
                                start=(ti == 0),
                                stop=(ti == len(TAPS) - 1),
                                tile_position=(32 * i, 32 * j),
                                # col groups share each bank (disjoint
                                # partition slices); the sim's group check
                                # is partition-coarse and false-positives.
                                skip_group_check=True,
                            )
                # evict: one f32->bf16 copy per band per step covering the
                # partition range of the active col groups
                if do_mm == 1:
                    np_hi = 32 * (max(jact) + 1)
                    for i in range(NBANDS):
                        dst = out_tiles[i][0:np_hi, s * W : (s + 1) * W]
                        if i % 2 == 0:
                            nc.vector.tensor_copy(dst, psums[i][0:np_hi, :])
                        else:
                            nc.scalar.copy(dst, psums[i][0:np_hi, :])

            # ---- store: out_tile partition 32j+c, free s*W+x maps to
            # out[b, c, row0 + starts[j] + s, x]: lens[j] consecutive rows.
            for i in range(NBANDS):
                row0 = i * HB + t * R
                for j in range(NCOLG):
                    src = out_tiles[i][
                        32 * j : 32 * j + 32, 0 : lens[j] * W
                    ].rearrange("c (s x) -> c s x", x=W)
                    r0 = row0 + starts[j]
                    nc.sync.dma_start(
                        out=out_ap[b, :, r0 : r0 + lens[j], :], in_=src
                    )


def build_conv_nc(B, C, H, W, R=32, passes=1, do_mm=True):
    nc = bass.Bass("TRN2", target_bir_lowering=False, debug=False)
    x = nc.declare_dram_parameter("x", [B, C, H, W], BF16, isOutput=False)
    w = nc.declare_dram_parameter("kernels_t", [C, KH * KW * C], BF16, isOutput=False)
    out = nc.declare_dram_parameter("out", [B, C, H, W], BF16, isOutput=True)
    with tile.TileContext(nc) as tc:
        with ExitStack() as ctx:
            emit_conv(nc, tc, ctx, x[:], w[:], out[:], B, C, H, W, R=R, do_mm=do_mm)
        # extra timing-probe passes into a scratch DRAM tensor (own pool
        # scope so SBUF is reused)
        for p in range(1, passes):
            scratch = nc.dram_tensor(f"scratch{p}", [B, C, H, W], BF16)
            with ExitStack() as ctx:
                emit_conv(
                    nc, tc, ctx, x[:], w[:], scratch[:], B, C, H, W, R=R,
                    prefix=f"p{p}_", do_mm=do_mm,
                )
    split_multi_waits(nc, cap=1)
    return nc


_NC_CACHE = {}


def _get_nc():
    key = (FULL_B // N_CORES, FULL_C, FULL_H, FULL_W)
    if key not in _NC_CACHE:
        _NC_CACHE[key] = build_conv_nc(*key)
    return _NC_CACHE[key]


def host_weights(kernels: np.ndarray) -> np.ndarray:
    # [oc, ic, kh, kw] -> [ic, (kh kw oc)] contiguous bf16, so the weight
    # DMA is a plain 2D copy.
    return np.ascontiguousarray(
        kernels.transpose(1, 2, 3, 0).reshape(32, -1)
    ).astype(NP_BF16)


def host_x(x: np.ndarray) -> np.ndarray:
    return np.asarray(x).astype(NP_BF16)


def kernel(x: np.ndarray, kernels: np.ndarray) -> np.ndarray:
    assert x.shape == (FULL_B, FULL_C, FULL_H, FULL_W), x.shape
    nc = _get_nc()
    bl = FULL_B // N_CORES
    wt = host_weights(np.asarray(kernels, dtype=np.float32))
    xs = host_x(x)
    in_maps = [
        {"x": xs[i * bl : (i + 1) * bl], "kernels_t": wt} for i in range(N_CORES)
    ]
    res = run_bass_kernel_spmd(nc, in_maps, list(range(N_CORES))).results
    out = np.concatenate([res[i]["out"] for i in range(N_CORES)], axis=0)
    return out.astype(np.float32)


# revision 25
# speedup vs baseline: 1.0105x; 1.0105x over previous
"""Conv2d 3x3 (stride 1, pad 1) on Trainium2, data-parallel over batch.

Full problem: x [16, 32, 512, 512] f32, kernels [32, 32, 3, 3] f32
-> out [16, 32, 512, 512] f32.

Sharding: batch 16 / 8 cores = 2 images per core; kernels replicated.
No collectives needed.

Per-core kernel strategy (memory-bound target):
- Host casts x and weights to bf16 (tol 2e-2; measured ~2e-3 rel err),
  halving both DMA directions. Output is written bf16 and upcast on host.
- Conv expressed as 9 accumulating 32x32 matmuls (one per tap) into PSUM.
- The 128x128 PE array is addressed as 16 concurrent 32x32 sub-arrays via
  tile_position: row group i = image band i (4 horizontal bands), col
  group j = output-row slot j.
- No padded input layout: width-edge taps (kw != 1) run as width-511
  matmuls into a column-shifted PSUM slice. The first tap is a kw=1
  full-width matmul with start=True so every PSUM element is initialized.
  This keeps every DMA fully contiguous per channel (large descriptors).
- Row mapping m = S*j + s: PSUM col group j holds output rows
  [row0+S*j, row0+S*j+S), so each output DMA writes S consecutive image
  rows = S KiB contiguous runs per channel.
"""

import numpy as np
from contextlib import ExitStack

import concourse.bass as bass
import concourse.mybir as mybir
import concourse.tile as tile
from concourse.bass_utils import run_bass_kernel_spmd

F32 = mybir.dt.float32
BF16 = mybir.dt.bfloat16
NP_BF16 = mybir.dt.np(BF16)
KH = KW = 3
NBANDS = 4  # row groups = horizontal image bands
NCOLG = 4  # col groups = output-row blocks per band

# Taps ordered so the first is kw=1 (full width, start=True initializes
# every PSUM column); kw=0/2 taps only touch a width-511 slice.
TAPS = [(0, 1), (1, 1), (2, 1), (0, 0), (1, 0), (2, 0), (0, 2), (1, 2), (2, 2)]

# Full-problem geometry (hardcoded; kernel.py must be self-contained)
FULL_B, FULL_C, FULL_H, FULL_W = 16, 32, 512, 512
N_CORES = 8


def split_multi_waits(nc, cap=1):
    """This walrus build rejects instructions carrying more than `cap` sync
    wait commands ("Too many sync wait commands", setupSyncWait). Hoist
    excess waits onto single-wait NoOps inserted just before the instruction
    on the same engine queue (queues are in-order, so semantics are
    unchanged)."""
    n_split = 0
    for fn in nc.m.functions:
        for blk in fn.blocks:
            insts = blk.instructions
            if not any(
                i.sync_info is not None and len(i.sync_info.on_wait) > cap
                for i in insts
            ):
                continue
            new = []
            for inst in insts:
                si = inst.sync_info
                if si is not None and len(si.on_wait) > cap:
                    waits = list(si.on_wait)
                    n_split += 1
                    for k in range(0, len(waits) - cap, cap):
                        nop = mybir.InstNoOp(
                            name=nc.get_next_instruction_name(), ins=[], outs=[]
                        )
                        nop.engine = inst.engine
                        nop.sync_info = mybir.SyncInfo(
                            on_wait=waits[k : k + cap], on_update=[]
                        )
                        new.append(nop)
                    inst.sync_info = mybir.SyncInfo(
                        on_wait=waits[len(waits) - cap :],
                        on_update=list(si.on_update),
                    )
                new.append(inst)
            blk.instructions = new
    return n_split


def emit_conv(
    nc, tc, ctx, x_ap, w_ap, out_ap, B, C, H, W, R=32, prefix="", do_mm=True,
    ncolg=NCOLG, fused_psum=0,
):
    """Emit the Tile program for a per-core conv: x [B, C, H, W] bf16
    (local batch), w [C, KH*KW*C] bf16 (pre-transposed on host:
    [ic, (kh kw oc)]), out [B, C, H, W] bf16."""
    assert C == 32
    HB = H // NBANDS  # rows per band
    assert HB * NBANDS == H
    assert HB % R == 0
    T = HB // R  # rounds per image
    # Split the R rows of a round across the 3 usable col groups as
    # contiguous blocks of lens[j] rows (group j owns rows
    # [starts[j], starts[j]+lens[j]) of the round).
    base_len = R // ncolg
    rem = R - base_len * ncolg
    lens = [base_len + (1 if j < rem else 0) for j in range(ncolg)]
    starts = [sum(lens[:j]) for j in range(ncolg)]
    S = lens[0]  # steps per round (group j idles for s >= lens[j])
    RH = R + 2  # input rows needed per round (1-row halo on each side)

    w_pool = ctx.enter_context(tc.tile_pool(name=prefix + "wpool", bufs=1))
    in_pool = ctx.enter_context(tc.tile_pool(name=prefix + "inpool", bufs=3))
    out_pool = ctx.enter_context(
        tc.tile_pool(name=prefix + "outpool", bufs=2 if fused_psum else 2 * NBANDS)
    )
    psum_pool = ctx.enter_context(
        tc.tile_pool(
            name=prefix + "psumpool",
            bufs=2 if fused_psum else 2 * NBANDS,
            space="PSUM",
        )
    )

    # Weights: replicate [32, 9*32] into each of the 4 partition groups so
    # lhsT.base_partition() matches the rhs row group.
    w_tile = w_pool.tile([128, KH * KW * C], BF16, name=prefix + "w_tile", tag="w")
    for r in range(NBANDS):
        nc.sync.dma_start(out=w_tile[32 * r : 32 * r + 32, :], in_=w_ap[:, :])

    for b in range(B):
        for t in range(T):
            # ---- load input rows for this round: band i covers output rows
            # [i*HB + t*R, i*HB + t*R + R), needing input rows -1..R+1
            # around it. Packed layout: slot r, col x contiguous.
            in_tile = in_pool.tile(
                [128, RH * W], BF16, name=f"{prefix}in_{b}_{t}", tag="xin"
            )
            in_rows = in_tile.rearrange("p (r w) -> p r w", w=W)
            for i in range(NBANDS):
                row0 = i * HB + t * R
                lo = max(row0 - 1, 0)
                hi = min(row0 + R + 1, H)
                slot0 = lo - (row0 - 1)
                cnt = hi - lo
                dst = in_rows[32 * i : 32 * i + 32, slot0 : slot0 + cnt, :]
                nc.sync.dma_start(out=dst, in_=x_ap[b, :, lo:hi, :])
                if row0 == 0:  # top image boundary: zero row slot 0
                    nc.vector.memset(in_rows[32 * i : 32 * i + 32, 0:1, :], 0.0)
                if row0 + R == H:  # bottom image boundary: zero last slot
                    nc.vector.memset(
                        in_rows[32 * i : 32 * i + 32, RH - 1 : RH, :], 0.0
                    )

            out_tiles = []
            if fused_psum:
                oround = out_pool.tile(
                    [128, NBANDS * S * W],
                    BF16,
                    name=f"{prefix}out_{b}_{t}",
                    tag="osb",
                )
                if do_mm != 1:
                    nc.vector.memset(oround[:, 0:1], 0.0)
                for i in range(NBANDS):
                    out_tiles.append(oround[:, i * S * W : (i + 1) * S * W])
            else:
                for i in range(NBANDS):
                    ot = out_pool.tile(
                        [128, S * W], BF16, name=f"{prefix}out_{b}_{t}_{i}", tag="osb"
                    )
                    if do_mm != 1:  # timing probe: mark tile written
                        nc.vector.memset(ot[:, 0:1], 0.0)
                    out_tiles.append(ot)

            for s in range(S if do_mm else 0):
                jact = [j for j in range(ncolg) if s < lens[j]]
                psums = {}
                if fused_psum:
                    # one 4-bank PSUM tile per step: single evict, 4x fewer
                    # semaphore hops on the PE stream
                    pf = psum_pool.tile(
                        [128, NBANDS * W],
                        F32,
                        name=f"{prefix}ps_{b}_{t}_{s}",
                        tag="acc",
                    )
                    for i in range(NBANDS):
                        psums[i] = pf[:, i * W : (i + 1) * W]
                else:
                    for i in range(NBANDS):
                        pt = psum_pool.tile(
                            [128, W], F32, name=f"{prefix}ps_{b}_{t}_{s}_{i}", tag="acc"
                        )
                        psums[i] = pt
                # 9 taps; 12 concurrent 32x32 sub-array matmuls per tap
                for ti, (kh, kw) in enumerate(TAPS):
                    off = kh * KW + kw
                    for i in range(NBANDS):
                        lhsT = w_tile[32 * i : 32 * i + 32, off * C : off * C + C]
                        for j in jact:
                            m = starts[j] + s  # local output row
                            base = (m + kh) * W  # input row slot m+kh
                            if kw == 1:
                                rhs = in_tile[32 * i : 32 * i + 32, base : base + W]
                                dst = psums[i][32 * j : 32 * j + 32, 0:W]
                            elif kw == 0:
                                rhs = in_tile[
                                    32 * i : 32 * i + 32, base : base + W - 1
                                ]
                                dst = psums[i][32 * j : 32 * j + 32, 1:W]
                            else:  # kw == 2
                                rhs = in_tile[
                                    32 * i : 32 * i + 32, base + 1 : base + W
                                ]
                                dst = psums[i][32 * j : 32 * j + 32, 0 : W - 1]
                            nc.tensor.matmul(
                                dst,
                                lhsT,
                                rhs,
                                start=(ti == 0),
                                stop=(ti == len(TAPS) - 1),
                                tile_position=(32 * i, 32 * j),
                                # col groups share each bank (disjoint
                                # partition slices); the sim's group check
                                # is partition-coarse and false-positives.
                                skip_group_check=True,
                            )
                # evict: one f32->bf16 copy per band per step covering the
                # partition range of the active col groups
                if do_mm == 1 and fused_psum:
                    np_hi = 32 * (max(jact) + 1)
                    # single strided evict per engine: src [p, 4*W] psum,
                    # dst band i at free i*S*W + s*W
                    dstv = oround.rearrange("p (i s x) -> p i s x", i=NBANDS, x=W)
                    half = np_hi // 64 * 32
                    nc.vector.tensor_copy(
                        dstv[0:half, :, s, :],
                        pf[0:half, :].rearrange("p (i x) -> p i x", x=W),
                    )
                    nc.scalar.copy(
                        dstv[half:np_hi, :, s, :],
                        pf[half:np_hi, :].rearrange("p (i x) -> p i x", x=W),
                    )
                elif do_mm == 1:
                    np_hi = 32 * (max(jact) + 1)
                    for i in range(NBANDS):
                        dst = out_tiles[i][0:np_hi, s * W : (s + 1) * W]
                        if i % 2 == 0:
                            nc.vector.tensor_copy(dst, psums[i][0:np_hi, :])
                        else:
                            nc.scalar.copy(dst, psums[i][0:np_hi, :])

            # ---- store: out_tile partition 32j+c, free s*W+x maps to
            # out[b, c, row0 + starts[j] + s, x]: lens[j] consecutive rows.
            for i in range(NBANDS):
                row0 = i * HB + t * R
                for j in range(ncolg):
                    src = out_tiles[i][
                        32 * j : 32 * j + 32, 0 : lens[j] * W
                    ].rearrange("c (s x) -> c s x", x=W)
                    r0 = row0 + starts[j]
                    nc.sync.dma_start(
                        out=out_ap[b, :, r0 : r0 + lens[j], :], in_=src
                    )


def build_conv_nc(
    B, C, H, W, R=32, passes=1, do_mm=True, ncolg=NCOLG, fused_psum=0
):
    nc = bass.Bass("TRN2", target_bir_lowering=False, debug=False)
    x = nc.declare_dram_parameter("x", [B, C, H, W], BF16, isOutput=False)
    w = nc.declare_dram_parameter("kernels_t", [C, KH * KW * C], BF16, isOutput=False)
    out = nc.declare_dram_parameter("out", [B, C, H, W], BF16, isOutput=True)
    with tile.TileContext(nc) as tc:
        with ExitStack() as ctx:
            emit_conv(
                nc, tc, ctx, x[:], w[:], out[:], B, C, H, W, R=R, do_mm=do_mm,
                ncolg=ncolg, fused_psum=fused_psum,
            )
        # extra timing-probe passes into a scratch DRAM tensor (own pool
        # scope so SBUF is reused)
        for p in range(1, passes):
            scratch = nc.dram_tensor(f"scratch{p}", [B, C, H, W], BF16)
            with ExitStack() as ctx:
                emit_conv(
                    nc, tc, ctx, x[:], w[:], scratch[:], B, C, H, W, R=R,
                    prefix=f"p{p}_", do_mm=do_mm, ncolg=ncolg,
                    fused_psum=fused_psum,
                )
    split_multi_waits(nc, cap=1)
    return nc


_NC_CACHE = {}


def _get_nc():
    key = (FULL_B // N_CORES, FULL_C, FULL_H, FULL_W)
    if key not in _NC_CACHE:
        _NC_CACHE[key] = build_conv_nc(*key)
    return _NC_CACHE[key]


def host_weights(kernels: np.ndarray) -> np.ndarray:
    # [oc, ic, kh, kw] -> [ic, (kh kw oc)] contiguous bf16, so the weight
    # DMA is a plain 2D copy.
    return np.ascontiguousarray(
        kernels.transpose(1, 2, 3, 0).reshape(32, -1)
    ).astype(NP_BF16)


def host_x(x: np.ndarray) -> np.ndarray:
    return np.asarray(x).astype(NP_BF16)


def kernel(x: np.ndarray, kernels: np.ndarray) -> np.ndarray:
    assert x.shape == (FULL_B, FULL_C, FULL_H, FULL_W), x.shape
    nc = _get_nc()
    bl = FULL_B // N_CORES
    wt = host_weights(np.asarray(kernels, dtype=np.float32))
    xs = host_x(x)
    in_maps = [
        {"x": xs[i * bl : (i + 1) * bl], "kernels_t": wt} for i in range(N_CORES)
    ]
    res = run_bass_kernel_spmd(nc, in_maps, list(range(N_CORES))).results
    out = np.concatenate([res[i]["out"] for i in range(N_CORES)], axis=0)
    return out.astype(np.float32)
